# revision 17
# baseline (speedup 1.0000x reference)
"""Multi-head attention Trainium2 kernel (8 NeuronCores).

Problem: B=2, S=4096, D=512, H=8 heads of dim 64.
Reference returns (output [B,S,D], attn_weights [B,H,S,S]).

Sharding: data-parallel over B (4 cores per batch) x tensor-parallel over
head-pairs (2 heads per core). Each core computes its 2 heads' full
attention matrix ([2,S,S] fp32, the dominant ~134MB HBM write) plus a
partial output projection; the host sums the 4 partials per batch.

Device algorithm per core (all matmuls float32r ~= tf32 precision):
  - QT/KT [128,S] and V [S,128] projections from host-pre-transposed X.
  - Two independent passes over the scores, interleaved per 1024-wide
    q-chunk so ScalarE / TensorE / DMA stay evenly loaded:
    * Pass 1 (orientation [k,q]): scores^T tiles on PE, transient
      exp(s/8) on ScalarE, V matmul accumulates unnormalized ctx^T.
    * Pass 2 (orientation [q,k]): scores tiles on PE, exp(s/8) on
      ScalarE with accum_out giving the softmax denominators per q
      (partition axis), reciprocal + in-place normalize on VectorE,
      DMA to HBM in natural [q,k] layout.
  - Epilogue: out_partial[q,512] = sum_h recip_h[q] * (ctx_h^T)^T @ w_o_h.
"""

import os

import numpy as np

import concourse.bass as bass
import concourse.mybir as mybir
import concourse.tile as tile
from concourse import bacc
from concourse.bass_utils import run_bass_kernel_spmd

F32 = mybir.dt.float32
F32R = mybir.dt.float32r

D_MODEL = 512
N_HEADS = 8
HEAD_DIM = 64
B = 2
S_FULL = 4096
N_CORES = 8
HP = 2  # heads per core


def build_kernel(S=S_FULL):
    """Build the per-core Bass kernel. Same program on all 8 cores."""
    assert S % 1024 == 0 or S in (256, 512)
    QCW = 1024 if S % 1024 == 0 else S  # q/k tile width
    KB = S // 128          # number of 128-row k blocks
    NQC = S // QCW         # number of q chunks
    QB = S // 128          # number of 128-row q blocks
    NKC = S // QCW         # number of k chunks per q block (pass 2)
    QBC = QCW // 128       # q blocks per q chunk

    P2W = min(int(os.environ.get("K_P2W", QCW)), S)  # pass-2 tile width
    NKC2 = S // P2W        # pass-2 k chunks per q block

    nc = bacc.Bacc("TRN2")

    # ---- DRAM I/O ----
    xqt = nc.dram_tensor("xqt", [D_MODEL, S], F32R, kind="ExternalInput")
    xkt = nc.dram_tensor("xkt", [D_MODEL, S], F32R, kind="ExternalInput")
    xvt = nc.dram_tensor("xvt", [D_MODEL, S], F32R, kind="ExternalInput")
    # [513, 128]: rows 0..511 = W[rows,:].T for this head pair, row 512 = bias
    wqt = nc.dram_tensor("wqt", [D_MODEL + 1, 128], F32R, kind="ExternalInput")
    wkt = nc.dram_tensor("wkt", [D_MODEL + 1, 128], F32R, kind="ExternalInput")
    wvt = nc.dram_tensor("wvt", [D_MODEL + 1, 128], F32R, kind="ExternalInput")
    # w_o columns for this head pair, transposed: [128, 512]
    wot = nc.dram_tensor("wot", [128, D_MODEL], F32R, kind="ExternalInput")

    attn = nc.dram_tensor("attn", [HP, S, S], F32, kind="ExternalOutput")
    outp = nc.dram_tensor("outp", [S, D_MODEL], F32, kind="ExternalOutput")

    KIN = D_MODEL // 128   # 4 input-dim tiles

    with tile.TileContext(nc) as tc:
        with (
            tc.tile_pool(name="persist", bufs=1) as persist,
            tc.tile_pool(name="ps1", bufs=1, space="PSUM") as ps1,
            tc.tile_pool(name="ps2", bufs=int(os.environ.get("K_PS2B", 2)),
                         space="PSUM") as ps2,
            tc.tile_pool(name="psC", bufs=1, space="PSUM") as psC,
        ):
            # ---- persistent tiles ----
            QT = persist.tile([128, S], F32R)      # rows 0-63 h0, 64-127 h1
            KT = persist.tile([128, S], F32R)
            V_h = [persist.tile([128, KB * 64], F32R, name=f"V{i}")
                   for i in range(HP)]
            ctxT = [persist.tile([64, S], F32R, name=f"ctxT{i}") for i in range(HP)]
            recipT = [persist.tile([128, QB], F32, name=f"recipT{i}")
                      for i in range(HP)]
            ones_row = persist.tile([1, 512], F32R)
            nc.vector.memset(ones_row.bitcast(F32), 1.0)

            # ---- load weights ----
            wq_sb, wk_sb, wv_sb = [], [], []
            for name, dram, lst in (("q", wqt, wq_sb), ("k", wkt, wk_sb), ("v", wvt, wv_sb)):
                for i in range(KIN):
                    t = persist.tile([128, 128], F32R, name=f"w{name}{i}", tag=f"w{name}{i}")
                    nc.sync.dma_start(t[:], dram[128 * i:128 * (i + 1), :])
                    lst.append(t)
                tb = persist.tile([1, 128], F32R, name=f"w{name}b", tag=f"w{name}b")
                nc.sync.dma_start(tb[:], dram[D_MODEL:D_MODEL + 1, :])
                lst.append(tb)
            wot_h = []
            for i in range(HP):
                t = persist.tile([64, D_MODEL], F32R, name=f"wo{i}", tag=f"wo{i}")
                nc.sync.dma_start(t[:], wot[64 * i:64 * (i + 1), :])
                wot_h.append(t)

            # ---- projections ----
            with tc.tile_pool(name="xstage", bufs=1) as xstage:
                for xdram, kind in ((xqt, "q"), (xkt, "k"), (xvt, "v")):
                    xt = [xstage.tile([128, S], F32R, name=f"x{i}", tag=f"x{i}")
                          for i in range(KIN)]
                    for i in range(KIN):
                        nc.sync.dma_start(xt[i][:], xdram[128 * i:128 * (i + 1), :])
                    w = {"q": wq_sb, "k": wk_sb, "v": wv_sb}[kind]
                    if kind in ("q", "k"):
                        dst = QT if kind == "q" else KT
                        for c in range(S // 512):
                            ps = ps2.tile([128, 512], F32, tag="p2")
                            for i in range(KIN):
                                nc.tensor.matmul(
                                    ps[:], w[i][:], xt[i][:, 512 * c:512 * (c + 1)],
                                    start=(i == 0), stop=False)
                            nc.tensor.matmul(
                                ps[:], w[KIN][:], ones_row[:],
                                start=False, stop=True)
                            nc.vector.tensor_copy(dst[:, 512 * c:512 * (c + 1)], ps[:])
                    else:
                        # V natural [tok, d]: lhsT = xT tile (X as weights)
                        for tb in range(KB):
                            ps = ps2.tile([128, 128], F32, tag="p2")
                            for i in range(KIN):
                                nc.tensor.matmul(
                                    ps[:], xt[i][:, 128 * tb:128 * (tb + 1)], w[i][:],
                                    start=(i == 0), stop=False)
                            nc.tensor.matmul(
                                ps[:], ones_row[:, 0:128], w[KIN][:],
                                start=False, stop=True)
                            for i in range(HP):
                                nc.vector.tensor_copy(
                                    V_h[i][:, 64 * tb:64 * (tb + 1)],
                                    ps[:, 64 * i:64 * (i + 1)])

            # ---- attention ----
            with (
                tc.tile_pool(name="p1exp", bufs=int(os.environ.get("K_EXB", 3))) as p1exp,
                tc.tile_pool(name="attnout", bufs=int(os.environ.get("K_AOB", 3))) as attnout,
                tc.tile_pool(name="accs", bufs=4) as accs,
                tc.tile_pool(name="opart", bufs=3) as opart,
            ):
                def p1_tile(h, qc, kb, ctx_ps):
                    """Pass-1 tile: scores^T [128k, QCW q] -> exp -> AV."""
                    qlo, qhi = 64 * h, 64 * (h + 1)
                    sps = ps1.tile([128, QCW], F32, tag="p1", name="sps1")
                    for half in range(QCW // 512):
                        sl = slice(512 * half, 512 * (half + 1))
                        nc.tensor.matmul(
                            sps[:, sl],
                            KT[qlo:qhi, 128 * kb:128 * (kb + 1)],
                            QT[qlo:qhi, QCW * qc + 512 * half:
                               QCW * qc + 512 * (half + 1)],
                            start=True, stop=True)
                    et = p1exp.tile([128, QCW], F32R, tag="exp", name="et")
                    nc.scalar.activation(
                        et[:], sps[:],
                        mybir.ActivationFunctionType.Exp, scale=0.125)
                    for half in range(QCW // 512):
                        sl = slice(512 * half, 512 * (half + 1))
                        nc.tensor.matmul(
                            ctx_ps[:, sl],
                            V_h[h][:, 64 * kb:64 * (kb + 1)],
                            et[:, sl],
                            start=(kb == 0), stop=(kb == KB - 1))

                def p2_tile(h, qb, kc, at, ac):
                    """Pass-2 tile: scores [128q, P2W k] -> exp(+accum)."""
                    qlo, qhi = 64 * h, 64 * (h + 1)
                    sps = ps2.tile([128, P2W], F32, tag="p2", name="sps2")
                    for half in range(P2W // 512):
                        sl = slice(512 * half, 512 * (half + 1))
                        nc.tensor.matmul(
                            sps[:, sl],
                            QT[qlo:qhi, 128 * qb:128 * (qb + 1)],
                            KT[qlo:qhi, P2W * kc + 512 * half:
                               P2W * kc + 512 * (half + 1)],
                            start=True, stop=True)
                    nc.scalar.activation(
                        at[:, P2W * kc:P2W * (kc + 1)], sps[:],
                        mybir.ActivationFunctionType.Exp,
                        scale=0.125, accum_out=ac[:, kc:kc + 1])

                def p2_finish(h, qb, at, ac):
                    if NKC2 > 1:
                        nc.vector.tensor_reduce(
                            recipT[h][:, qb:qb + 1], ac[:, 0:NKC2],
                            axis=mybir.AxisListType.X,
                            op=mybir.AluOpType.add)
                        nc.vector.reciprocal(
                            recipT[h][:, qb:qb + 1], recipT[h][:, qb:qb + 1])
                    else:
                        nc.vector.reciprocal(recipT[h][:, qb:qb + 1], ac[:, 0:1])
                    nc.vector.tensor_scalar_mul(
                        at[:], at[:], recipT[h][:, qb:qb + 1])
                    nc.sync.dma_start(
                        attn[h, 128 * qb:128 * (qb + 1), :], at[:])

                for h in range(HP):
                    for qc in range(NQC):
                        # Interleave pass-1 (kb tiles) with pass-2 tiles of
                        # the same chunk: the streams are independent, so
                        # PE/ACT/DVE/DMA stay uniformly busy.
                        p2q = [(qc * QBC + qi, kc)
                               for qi in range(QBC) for kc in range(NKC2)]
                        if os.environ.get("K_SEQ", "0") == "1":
                            every = KB + 1
                        else:
                            every = max(1, KB // max(len(p2q), 1))
                        ctx_ps = psC.tile([64, QCW], F32, tag="ctx", name="ctx_ps")
                        cur = {}  # qb -> (at, ac)
                        p2i = 0

                        def emit_p2(p2i):
                            qb, kc = p2q[p2i]
                            if kc == 0:
                                cur[qb] = (
                                    attnout.tile([128, S], F32, tag="attn",
                                                 name="at"),
                                    accs.tile([128, max(NKC2, 2)], F32,
                                              tag="ac", name="ac"),
                                )
                            at, ac = cur[qb]
                            p2_tile(h, qb, kc, at, ac)
                            if kc == NKC2 - 1:
                                p2_finish(h, qb, at, ac)
                                del cur[qb]
                            return p2i + 1

                        for kb in range(KB):
                            p1_tile(h, qc, kb, ctx_ps)
                            if (kb + 1) % every == 0 and p2i < len(p2q):
                                p2i = emit_p2(p2i)
                        while p2i < len(p2q):
                            p2i = emit_p2(p2i)
                        nc.vector.tensor_copy(
                            ctxT[h][:, QCW * qc:QCW * (qc + 1)], ctx_ps[:])

                # ---- epilogue: output projection ----
                # accumulate GRP q-blocks into one SBUF tile -> 1 DMA each
                GRP = min(4, QB)
                outp_v = outp.rearrange("(g b) d -> g b d", b=128 * GRP)
                for qg in range(QB // GRP):
                    acc = opart.tile([128, GRP * D_MODEL], F32, tag="opacc")
                    for qi in range(GRP):
                        qb = qg * GRP + qi
                        osl = slice(D_MODEL * qi, D_MODEL * (qi + 1))
                        for h in range(HP):
                            ops = ps2.tile([128, D_MODEL], F32, tag="p2")
                            nc.tensor.matmul(
                                ops[:], ctxT[h][:, 128 * qb:128 * (qb + 1)],
                                wot_h[h][:], start=True, stop=True)
                            if h == 0:
                                nc.vector.tensor_scalar_mul(
                                    acc[:, osl], ops[:], recipT[h][:, qb:qb + 1])
                            else:
                                ot = opart.tile([128, D_MODEL], F32, tag="oph")
                                nc.vector.tensor_scalar_mul(
                                    ot[:], ops[:], recipT[h][:, qb:qb + 1])
                                nc.vector.tensor_add(
                                    acc[:, osl], acc[:, osl], ot[:])
                    dst = outp_v[qg].rearrange("(b p) d -> p b d", p=128)
                    src = acc[:].rearrange("p (b d) -> p b d", d=D_MODEL)
                    nc.sync.dma_start(dst, src)

    nc.compile()
    return nc


_NC_CACHE = {}


def _get_kernel(S):
    if S not in _NC_CACHE:
        _NC_CACHE[S] = build_kernel(S)
    return _NC_CACHE[S]


def _make_in_maps(inputs):
    query = np.asarray(inputs["query"], dtype=np.float32)
    key = np.asarray(inputs["key"], dtype=np.float32)
    value = np.asarray(inputs["value"], dtype=np.float32)
    w_q, b_q = np.asarray(inputs["w_q"], np.float32), np.asarray(inputs["b_q"], np.float32)
    w_k, b_k = np.asarray(inputs["w_k"], np.float32), np.asarray(inputs["b_k"], np.float32)
    w_v, b_v = np.asarray(inputs["w_v"], np.float32), np.asarray(inputs["b_v"], np.float32)
    w_o = np.asarray(inputs["w_o"], np.float32)

    b = query.shape[0]
    xq_t = [np.ascontiguousarray(query[i].T) for i in range(b)]
    xk_t = [np.ascontiguousarray(key[i].T) for i in range(b)]
    xv_t = [np.ascontiguousarray(value[i].T) for i in range(b)]

    in_maps = []
    for c in range(N_CORES):
        bi, hp = c // 4, c % 4
        rows = slice(128 * hp, 128 * (hp + 1))
        in_maps.append({
            "xqt": xq_t[bi], "xkt": xk_t[bi], "xvt": xv_t[bi],
            "wqt": np.ascontiguousarray(
                np.concatenate([w_q[rows].T, b_q[rows][None, :]], axis=0)),
            "wkt": np.ascontiguousarray(
                np.concatenate([w_k[rows].T, b_k[rows][None, :]], axis=0)),
            "wvt": np.ascontiguousarray(
                np.concatenate([w_v[rows].T, b_v[rows][None, :]], axis=0)),
            "wot": np.ascontiguousarray(w_o[:, rows].T),
        })
    return in_maps


def kernel(query, key, value, w_q, b_q, w_k, b_k, w_v, b_v, w_o, b_o):
    inputs = {
        "query": query, "key": key, "value": value,
        "w_q": w_q, "b_q": b_q, "w_k": w_k, "b_k": b_k,
        "w_v": w_v, "b_v": b_v, "w_o": w_o, "b_o": b_o,
    }
    query = np.asarray(query, dtype=np.float32)
    b_o = np.asarray(b_o, np.float32)
    b, s, _ = query.shape
    nc = _get_kernel(s)
    in_maps = _make_in_maps(inputs)

    res = run_bass_kernel_spmd(nc, in_maps, core_ids=list(range(N_CORES)))

    attn_full = np.empty((b, N_HEADS, s, s), dtype=np.float32)
    out_full = np.zeros((b, s, D_MODEL), dtype=np.float32)
    for c in range(N_CORES):
        bi, hp = c // 4, c % 4
        attn_full[bi, 2 * hp] = res.results[c]["attn"][0]
        attn_full[bi, 2 * hp + 1] = res.results[c]["attn"][1]
        out_full[bi] += res.results[c]["outp"]
    out_full += b_o
    return out_full, attn_full
